# revision 7
# baseline (speedup 1.0000x reference)
"""Trainium2 Bass kernel for 2D Haar DWT (single-level, reflect-pad quirk irrelevant
for even H/W) matching the reference DWT2D_Haar module.

Full input:  x (8, 64, 512, 512) f32
Full output: tuple (LL, LH, HL, HH), each (8, 64, 256, 256) f32, where the
             "subbands" are contiguous quarters of the channel-interleaved
             grouped-conv output (out channel = 4*c + s).

Sharding: pure data parallel over batch — core i handles x[i].

Per-core kernel (64 channels of 512x512):
  - tile = 2 channels, loaded as one contiguous 2 MiB DMA into [128, 4096] f32
    (partition p holds 8 consecutive image rows = 4 row-pairs)
  - ACT engine prescales by 0.5 in place (exact, power of two)
  - DVE row butterfly: S = Xe+Xo, D = Xe-Xo  (rows adjacent in free dim)
  - DVE col butterfly: ll/lh/hl/hh from stride-2 column pairs, written directly
    into the interleaved-subband output layout
  - one 2 MiB strided store to y[c, s, rp, col]
"""

import numpy as np

B, C, H, W = 8, 64, 512, 512
H2, W2 = H // 2, W // 2
N_CORES = 8
CH_PER_TILE = 2                      # channels per SBUF tile
ROWS_PER_PART = CH_PER_TILE * H // 128   # 8 rows -> 4 row-pairs per partition
RP_PER_PART = ROWS_PER_PART // 2         # 4
FREE = ROWS_PER_PART * W                 # 4096 f32 per partition

_NC_CACHE = {}


def _build_nc():
    """Build the single-core Bass/Tile program (SPMD: same NEFF on all cores)."""
    from contextlib import ExitStack

    import concourse.bacc as bacc
    import concourse.mybir as mybir
    import concourse.tile as tile

    dt = mybir.dt.float32
    # Bacc (not plain Bass): its finalize() runs generate_event_semaphores,
    # which splits multi-wait DMAs into EventSemaphore + 1-wait instructions
    # (TRN2 ISA allows at most one embedded wait per instruction).
    nc = bacc.Bacc("TRN2", target_bir_lowering=False, debug=False)
    x = nc.declare_dram_parameter("x", [C, H, W], dt, isOutput=False)
    y = nc.declare_dram_parameter("y", [C, 4, H2, W2], dt, isOutput=True)

    n_tiles = C // CH_PER_TILE
    p_per_ch = 128 // CH_PER_TILE  # partitions per channel

    with tile.TileContext(nc) as tc, ExitStack() as ctx:
        xpool = ctx.enter_context(tc.tile_pool(name="x", bufs=3))
        xspool = ctx.enter_context(tc.tile_pool(name="xs", bufs=2))
        spool = ctx.enter_context(tc.tile_pool(name="s", bufs=2))
        dpool = ctx.enter_context(tc.tile_pool(name="d", bufs=2))
        opool = ctx.enter_context(tc.tile_pool(name="o", bufs=3))

        for t in range(n_tiles):
            c0 = t * CH_PER_TILE

            xt = xpool.tile([128, FREE], dt)
            # contiguous load: channels c0..c0+1, partition = 8 consecutive rows
            src = x[c0 : c0 + CH_PER_TILE].rearrange(
                "c (p q) w -> (c p) (q w)", p=p_per_ch
            )
            nc.gpsimd.dma_start(out=xt[:], in_=src)

            # prescale by 0.5 on ACT (separate dst tile keeps the load DMA's
            # wait list at one condition)
            xs = xspool.tile([128, FREE], dt)
            nc.scalar.mul(xs[:], xt[:], 0.5)

            # row butterfly: per partition free layout [b=4 rowpairs][r=2][w=512]
            xv = xs[:].rearrange("p (b r w) -> p b r w", b=RP_PER_PART, r=2)
            st = spool.tile([128, RP_PER_PART * W2 * 2], dt)  # [128, 2048]
            dtile = dpool.tile([128, RP_PER_PART * W2 * 2], dt)
            sv = st[:].rearrange("p (b w) -> p b w", b=RP_PER_PART)
            dv = dtile[:].rearrange("p (b w) -> p b w", b=RP_PER_PART)
            nc.vector.tensor_add(sv, xv[:, :, 0, :], xv[:, :, 1, :])
            nc.vector.tensor_sub(dv, xv[:, :, 0, :], xv[:, :, 1, :])

            # column butterfly: stride-2 pairs along w
            s2 = st[:].rearrange("p (b w q) -> p b w q", b=RP_PER_PART, q=2)
            d2 = dtile[:].rearrange("p (b w q) -> p b w q", b=RP_PER_PART, q=2)
            ot = opool.tile([128, 4 * RP_PER_PART * W2], dt)  # [128, 4096]
            ov = ot[:].rearrange("p (s b w) -> p s b w", s=4, b=RP_PER_PART)
            nc.vector.tensor_add(ov[:, 0], s2[:, :, :, 0], s2[:, :, :, 1])  # ll
            nc.vector.tensor_sub(ov[:, 1], s2[:, :, :, 0], s2[:, :, :, 1])  # lh
            nc.vector.tensor_add(ov[:, 2], d2[:, :, :, 0], d2[:, :, :, 1])  # hl
            nc.vector.tensor_sub(ov[:, 3], d2[:, :, :, 0], d2[:, :, :, 1])  # hh

            # store: y[c, s, rp, col]; partition p covers rp 4*(p%64)..+3 of
            # channel c0 + p//64. One DMA per channel (DMA APs cap at 3 dims).
            for j in range(CH_PER_TILE):
                dst = y[c0 + j].rearrange("s (p b) w -> p s (b w)", b=RP_PER_PART)
                nc.gpsimd.dma_start(
                    out=dst, in_=ot[j * p_per_ch : (j + 1) * p_per_ch, :]
                )

    nc.finalize()
    return nc


def _run(x: np.ndarray, trace: bool = False):
    """Run on 8 cores. Returns (y_full (8,64,4,256,256), BassKernelResults)."""
    from concourse.bass_utils import run_bass_kernel_spmd

    if "nc" not in _NC_CACHE:
        _NC_CACHE["nc"] = _build_nc()
    nc = _NC_CACHE["nc"]

    x = np.asarray(x, dtype=np.float32)
    in_maps = [{"x": x[i]} for i in range(N_CORES)]
    res = run_bass_kernel_spmd(
        nc, in_maps, list(range(N_CORES)), trace=trace
    )
    y = np.stack([res.results[i]["y"] for i in range(N_CORES)], axis=0)
    return y, res


def kernel(x: np.ndarray):
    y, _ = _run(x, trace=False)
    # y: (8, 64, 4, 256, 256) with out-channel = 4*c + s -> (8, 256, 256, 256)
    y = y.reshape(B, 4 * C, H2, W2)
    LL = y[:, 0 * C : 1 * C]
    LH = y[:, 1 * C : 2 * C]
    HL = y[:, 2 * C : 3 * C]
    HH = y[:, 3 * C : 4 * C]
    return (LL, LH, HL, HH)


# revision 9
# speedup vs baseline: 1.0030x; 1.0030x over previous
"""Trainium2 Bass kernel for 2D Haar DWT (single-level) matching the reference
DWT2D_Haar module.

Full input:  x (8, 64, 512, 512) f32
Full output: tuple (LL, LH, HL, HH), each (8, 64, 256, 256) f32, where the
             "subbands" are contiguous quarters of the channel-interleaved
             grouped-conv output (out channel = 4*c + s).

Sharding: pure data parallel over batch — core i handles x[i].

Per-core kernel (64 channels of 512x512):
  - tile = 2 channels, loaded as one contiguous 2 MiB DMA into [128, 4096] f32
    (partition p holds 8 consecutive image rows = 4 row-pairs)
  - ACT engine prescales by 0.5 in place (exact, power of two)
  - DVE row butterfly: S = Xe+Xo, D = Xe-Xo  (rows adjacent in free dim)
  - DVE col butterfly: ll/lh/hl/hh from stride-2 column pairs, written directly
    into the interleaved-subband output layout
  - one full-width 2 MiB strided store to y[c, s, rp, col] (the (c,p) partition
    merge is exact: channel stride 262144 = 64 partitions x 4096)
"""

import numpy as np

B, C, H, W = 8, 64, 512, 512
H2, W2 = H // 2, W // 2
N_CORES = 8
CH_PER_TILE = 2                      # channels per SBUF tile
ROWS_PER_PART = CH_PER_TILE * H // 128   # 8 rows -> 4 row-pairs per partition
RP_PER_PART = ROWS_PER_PART // 2         # 4
FREE = ROWS_PER_PART * W                 # 4096 f32 per partition

_NC_CACHE = {}


def _build_nc():
    """Build the single-core Bass/Tile program (SPMD: same NEFF on all cores)."""
    from contextlib import ExitStack

    import concourse.bacc as bacc
    import concourse.mybir as mybir
    import concourse.tile as tile

    dt = mybir.dt.float32
    # Bacc (not plain Bass): its finalize() runs generate_event_semaphores,
    # which splits multi-wait DMAs into EventSemaphore + 1-wait instructions
    # (TRN2 ISA allows at most one embedded wait per instruction).
    nc = bacc.Bacc("TRN2", target_bir_lowering=False, debug=False)
    x = nc.declare_dram_parameter("x", [C, H, W], dt, isOutput=False)
    y = nc.declare_dram_parameter("y", [C, 4, H2, W2], dt, isOutput=True)

    n_tiles = C // CH_PER_TILE
    p_per_ch = 128 // CH_PER_TILE  # partitions per channel

    with tile.TileContext(nc) as tc, ExitStack() as ctx:
        xpool = ctx.enter_context(tc.tile_pool(name="x", bufs=4))
        spool = ctx.enter_context(tc.tile_pool(name="s", bufs=3))
        dpool = ctx.enter_context(tc.tile_pool(name="d", bufs=3))
        opool = ctx.enter_context(tc.tile_pool(name="o", bufs=4))

        for t in range(n_tiles):
            c0 = t * CH_PER_TILE

            xt = xpool.tile([128, FREE], dt)
            # contiguous load: channels c0..c0+1, partition = 8 consecutive rows
            src = x[c0 : c0 + CH_PER_TILE].rearrange(
                "c (p q) w -> (c p) (q w)", p=p_per_ch
            )
            nc.sync.dma_start(out=xt[:], in_=src)

            # prescale by 0.5 on ACT, in place
            nc.scalar.mul(xt[:], xt[:], 0.5)

            # row butterfly: per partition free layout [b=4 rowpairs][r=2][w=512]
            xv = xt[:].rearrange("p (b r w) -> p b r w", b=RP_PER_PART, r=2)
            st = spool.tile([128, RP_PER_PART * W2 * 2], dt)  # [128, 2048]
            dtile = dpool.tile([128, RP_PER_PART * W2 * 2], dt)
            sv = st[:].rearrange("p (b w) -> p b w", b=RP_PER_PART)
            dv = dtile[:].rearrange("p (b w) -> p b w", b=RP_PER_PART)
            nc.vector.tensor_add(sv, xv[:, :, 0, :], xv[:, :, 1, :])
            nc.vector.tensor_sub(dv, xv[:, :, 0, :], xv[:, :, 1, :])

            # column butterfly: stride-2 pairs along w
            s2 = st[:].rearrange("p (b w q) -> p b w q", b=RP_PER_PART, q=2)
            d2 = dtile[:].rearrange("p (b w q) -> p b w q", b=RP_PER_PART, q=2)
            ot = opool.tile([128, 4 * RP_PER_PART * W2], dt)  # [128, 4096]
            ov = ot[:].rearrange("p (s b w) -> p s b w", s=4, b=RP_PER_PART)
            nc.vector.tensor_add(ov[:, 0], s2[:, :, :, 0], s2[:, :, :, 1])  # ll
            nc.vector.tensor_sub(ov[:, 1], s2[:, :, :, 0], s2[:, :, :, 1])  # lh
            nc.vector.tensor_add(ov[:, 2], d2[:, :, :, 0], d2[:, :, :, 1])  # hl
            nc.vector.tensor_sub(ov[:, 3], d2[:, :, :, 0], d2[:, :, :, 1])  # hh

            # store: y[c, s, rp, col]; partition p covers rp 4*(p%64)..+3 of
            # channel c0 + p//64. One DMA per channel (DMA APs cap at 3 dims;
            # the per-subband scatter keeps (c p) from merging).
            for j in range(CH_PER_TILE):
                dst = y[c0 + j].rearrange("s (p b) w -> p s (b w)", b=RP_PER_PART)
                nc.sync.dma_start(
                    out=dst, in_=ot[j * p_per_ch : (j + 1) * p_per_ch, :]
                )

    nc.finalize()
    return nc


def _run(x: np.ndarray, trace: bool = False):
    """Run on 8 cores. Returns (y_full (8,64,4,256,256), BassKernelResults)."""
    from concourse.bass_utils import run_bass_kernel_spmd

    if "nc" not in _NC_CACHE:
        _NC_CACHE["nc"] = _build_nc()
    nc = _NC_CACHE["nc"]

    x = np.asarray(x, dtype=np.float32)
    in_maps = [{"x": x[i]} for i in range(N_CORES)]
    res = run_bass_kernel_spmd(
        nc, in_maps, list(range(N_CORES)), trace=trace
    )
    y = np.stack([res.results[i]["y"] for i in range(N_CORES)], axis=0)
    return y, res


def kernel(x: np.ndarray):
    y, _ = _run(x, trace=False)
    # y: (8, 64, 4, 256, 256) with out-channel = 4*c + s -> (8, 256, 256, 256)
    y = y.reshape(B, 4 * C, H2, W2)
    LL = y[:, 0 * C : 1 * C]
    LH = y[:, 1 * C : 2 * C]
    HL = y[:, 2 * C : 3 * C]
    HH = y[:, 3 * C : 4 * C]
    return (LL, LH, HL, HH)


# revision 10
# speedup vs baseline: 1.4723x; 1.4679x over previous
"""Trainium2 Bass kernel for 2D Haar DWT (single-level) matching the reference
DWT2D_Haar module.

Full input:  x (8, 64, 512, 512) f32
Full output: tuple (LL, LH, HL, HH), each (8, 64, 256, 256) f32, where the
             "subbands" are contiguous quarters of the channel-interleaved
             grouped-conv output (out channel = 4*c + s).

Sharding: pure data parallel over batch — core i handles x[i].

Per-core kernel (64 channels of 512x512):
  - tile = 2 channels, loaded as one contiguous 2 MiB DMA into [128, 4096] f32
    (partition p holds 8 consecutive image rows = 4 row-pairs)
  - ACT engine prescales by 0.5 in place (exact, power of two)
  - DVE row butterfly: S = Xe+Xo, D = Xe-Xo  (rows adjacent in free dim)
  - DVE col butterfly: ll/lh/hl/hh from stride-2 column pairs, written directly
    into the interleaved-subband output layout
  - one full-width 2 MiB strided store to y[c, s, rp, col] (the (c,p) partition
    merge is exact: channel stride 262144 = 64 partitions x 4096)
"""

import numpy as np

B, C, H, W = 8, 64, 512, 512
H2, W2 = H // 2, W // 2
N_CORES = 8
CH_PER_TILE = 2                      # channels per SBUF tile
ROWS_PER_PART = CH_PER_TILE * H // 128   # 8 rows -> 4 row-pairs per partition
RP_PER_PART = ROWS_PER_PART // 2         # 4
FREE = ROWS_PER_PART * W                 # 4096 f32 per partition

_NC_CACHE = {}


def _build_nc():
    """Build the single-core Bass/Tile program (SPMD: same NEFF on all cores)."""
    from contextlib import ExitStack

    import concourse.bacc as bacc
    import concourse.mybir as mybir
    import concourse.tile as tile

    dt = mybir.dt.float32
    # Bacc (not plain Bass): its finalize() runs generate_event_semaphores,
    # which splits multi-wait DMAs into EventSemaphore + 1-wait instructions
    # (TRN2 ISA allows at most one embedded wait per instruction).
    nc = bacc.Bacc("TRN2", target_bir_lowering=False, debug=False)
    x = nc.declare_dram_parameter("x", [C, H, W], dt, isOutput=False)
    y = nc.declare_dram_parameter("y", [C, 4, H2, W2], dt, isOutput=True)

    n_tiles = C // CH_PER_TILE
    p_per_ch = 128 // CH_PER_TILE  # partitions per channel

    with tile.TileContext(nc) as tc, ExitStack() as ctx:
        xpool = ctx.enter_context(tc.tile_pool(name="x", bufs=4))
        spool = ctx.enter_context(tc.tile_pool(name="s", bufs=3))
        dpool = ctx.enter_context(tc.tile_pool(name="d", bufs=3))
        opool = ctx.enter_context(tc.tile_pool(name="o", bufs=4))

        for t in range(n_tiles):
            c0 = t * CH_PER_TILE

            xt = xpool.tile([128, FREE], dt)
            # contiguous load: channels c0..c0+1, partition = 8 consecutive rows
            src = x[c0 : c0 + CH_PER_TILE].rearrange(
                "c (p q) w -> (c p) (q w)", p=p_per_ch
            )
            nc.sync.dma_start(out=xt[:], in_=src)

            # prescale by 0.5 on ACT, in place
            nc.scalar.mul(xt[:], xt[:], 0.5)

            # row butterfly: per partition free layout [b=4 rowpairs][r=2][w=512]
            xv = xt[:].rearrange("p (b r w) -> p b r w", b=RP_PER_PART, r=2)
            st = spool.tile([128, RP_PER_PART * W2 * 2], dt)  # [128, 2048]
            dtile = dpool.tile([128, RP_PER_PART * W2 * 2], dt)
            sv = st[:].rearrange("p (b w) -> p b w", b=RP_PER_PART)
            dv = dtile[:].rearrange("p (b w) -> p b w", b=RP_PER_PART)
            nc.vector.tensor_add(sv, xv[:, :, 0, :], xv[:, :, 1, :])
            nc.vector.tensor_sub(dv, xv[:, :, 0, :], xv[:, :, 1, :])

            # column butterfly: stride-2 pairs along w
            s2 = st[:].rearrange("p (b w q) -> p b w q", b=RP_PER_PART, q=2)
            d2 = dtile[:].rearrange("p (b w q) -> p b w q", b=RP_PER_PART, q=2)
            ot = opool.tile([128, 4 * RP_PER_PART * W2], dt)  # [128, 4096]
            ov = ot[:].rearrange("p (s b w) -> p s b w", s=4, b=RP_PER_PART)
            nc.vector.tensor_add(ov[:, 0], s2[:, :, :, 0], s2[:, :, :, 1])  # ll
            nc.vector.tensor_sub(ov[:, 1], s2[:, :, :, 0], s2[:, :, :, 1])  # lh
            nc.vector.tensor_add(ov[:, 2], d2[:, :, :, 0], d2[:, :, :, 1])  # hl
            nc.vector.tensor_sub(ov[:, 3], d2[:, :, :, 0], d2[:, :, :, 1])  # hh

            # store: y[c, s, rp, col]; partition p covers rp 4*(p%64)..+3 of
            # channel c0 + p//64. One DMA per channel (DMA APs cap at 3 dims;
            # the per-subband scatter keeps (c p) from merging).
            for j in range(CH_PER_TILE):
                dst = y[c0 + j].rearrange("s (p b) w -> p s (b w)", b=RP_PER_PART)
                nc.scalar.dma_start(
                    out=dst, in_=ot[j * p_per_ch : (j + 1) * p_per_ch, :]
                )

    nc.finalize()
    return nc


def _run(x: np.ndarray, trace: bool = False):
    """Run on 8 cores. Returns (y_full (8,64,4,256,256), BassKernelResults)."""
    from concourse.bass_utils import run_bass_kernel_spmd

    if "nc" not in _NC_CACHE:
        _NC_CACHE["nc"] = _build_nc()
    nc = _NC_CACHE["nc"]

    x = np.asarray(x, dtype=np.float32)
    in_maps = [{"x": x[i]} for i in range(N_CORES)]
    res = run_bass_kernel_spmd(
        nc, in_maps, list(range(N_CORES)), trace=trace
    )
    y = np.stack([res.results[i]["y"] for i in range(N_CORES)], axis=0)
    return y, res


def kernel(x: np.ndarray):
    y, _ = _run(x, trace=False)
    # y: (8, 64, 4, 256, 256) with out-channel = 4*c + s -> (8, 256, 256, 256)
    y = y.reshape(B, 4 * C, H2, W2)
    LL = y[:, 0 * C : 1 * C]
    LH = y[:, 1 * C : 2 * C]
    HL = y[:, 2 * C : 3 * C]
    HH = y[:, 3 * C : 4 * C]
    return (LL, LH, HL, HH)
